# revision 28
# baseline (speedup 1.0000x reference)
"""Multi-head masked self-attention on 8 TRN2 NeuronCores.

Problem: B=4, S=2048, D=1024, H=16 heads (hd=64), fp32.
  q,k,v = x@W* + b*; causal softmax(q k^T / 8) @ v; out = ctx @ Wo + bo.

Sharding: core c -> (batch b = c//2, head-group g = c%2 of 8 heads).
Each core computes a partial output projection over its 512 hidden dims;
the host sums the two partials per batch and adds bo.

On-device layout strategy (no on-device transposes needed):
  - host passes xT = x[b].T  [D, S]
  - q^T, k^T computed directly as [512, S] (lhsT = W chunk, rhs = xT chunk)
  - v computed in natural [S, 512] layout (lhsT = xT chunk, rhs = Wv chunk),
    stored interleaved with 64 ones-columns per head ("v_aug", [S, 8*128]):
    the ctx matmul then accumulates the softmax denominator REPLICATED on
    PSUM partitions 64:128 for free (matmul cost depends only on the
    streamed column count N, not the stationary width M), so normalization
    is just a [64,512] reciprocal + multiply on DVE -- no gpsimd
    partition_broadcast, no denominator copy
  - scores are computed transposed: st[sk, sq] = k q^T; exp via ACT with the
    free affine bias: e = exp(s/8 - 2).  The -2 keeps e inside fp16 range
    (max score/8 measured ~8.8 -> e^6.8 ~ 900) and cancels exactly in the
    softmax normalization (numerator and denominator share the factor).
  - every diagonal chunk t=ik-4j in {0,1,2,3} computes only the columns the
    causal boundary allows (cq = 128*t), and in that frame the boundary
    always cuts through relative columns 0:128 with the SAME pattern
    (c >= p), so one [128,128] mask serves all four chunk types
  - heads are processed in PAIRS (partitions 0:64 / 64:128): the two K=64
    score matmuls per sk-chunk share one PSUM tile (separate banks) and are
    adjacent in program order, so the PE runs them concurrently on disjoint
    row-groups (tile_position (0,0)/(64,0)) -- 2x score throughput
  - ctx^T[hd, sq] accumulated in PSUM = v_aug^T.T @ exp; normalization:
    fast approximate reciprocal of the den row on DVE, gpsimd
    partition_broadcast, multiply on DVE during evacuation
  - output projection uses ctx^T directly as lhsT (again no transpose);
    ctx^T aliases qT's storage (each qT j-tile dies as its wave completes)
  - all matmul operands are float16 (true 1 col/cycle streaming + FWL
    weight loads, unlike f32r whose fp32_mode=HIGH path measures ~1.8x
    slower per matmul and disables FWL); accumulation stays fp32 in PSUM.
    fp16 keeps ~11 bits of mantissa -> rel err ~1e-3, far under the 2e-2
    gate.
  - PSUM pools: scores/C 2x[128,2,512], A-chains 2x[128,512] (dedicated so
    pool rotation never lets attention stall the projections), ctx 2x[65,512]
  - stage A(j+1) is cut into 12 half-chain units and WOVEN between wave j's
    score chunks (Feeder); likewise C(sq<12) weaves into wave 3.  This keeps
    independent PE work queued behind every exp-gated ctx matmul, which keeps
    PE duty high enough that the HAM clock stays at 2.4 GHz for the whole
    middle of the kernel (HAM re-throttles to 1.2 GHz after idle windows)
"""

import numpy as np

import concourse.bass as bass
import concourse.mybir as mybir
import concourse.tile as tile
from concourse import bacc
from concourse.bass import ts
from concourse.bass_utils import run_bass_kernel_spmd

F32 = mybir.dt.float32
F32R = mybir.dt.float32r
F16 = mybir.dt.float16
AF = mybir.ActivationFunctionType

B, S, D, H, HD = 4, 2048, 1024, 16, 64
G = 2                 # head groups (cores per batch)
DH = D // G           # hidden dims per core = 512
HPC = H // G          # heads per core = 8
NCORES = 8

NSQ = S // 512        # 4 sq tiles of 512
NSK = S // 128        # 16 sk chunks of 128
NFC = D // 128        # 8 feature chunks
NOC = DH // 128       # 4 out-dim chunks of the per-core hidden

EXP_BIAS = -2.0       # e = exp(s/8 + EXP_BIAS); cancels in normalization

_DT = {"fp16": F16, "f32r": F32R, "f32": F32}
_NPDT = {"fp16": np.float16, "f32r": np.float32, "f32": np.float32}


def _mm(nc, out, lhsT, rhs, start, stop):
    nc.tensor.matmul(out, lhsT, rhs, start=start, stop=stop)


def build_program(mode="fp16"):
    """Build the single-core SPMD Bass program (same program on all 8 cores)."""
    nc = bacc.Bacc("TRN2", target_bir_lowering=False, debug=False)
    MMDT = _DT[mode]  # dtype of every matmul operand

    # all large inputs arrive pre-tiled by the host into the exact SBUF
    # layout, so every load is one fully-contiguous DMA at line rate
    xT_d = nc.dram_tensor("xT", [NSQ, 128, NFC, 512], MMDT,
                          kind="ExternalInput").ap()
    wq_d = nc.dram_tensor("wq", [128, NFC, DH], MMDT, kind="ExternalInput").ap()
    wk_d = nc.dram_tensor("wk", [128, NFC, DH], MMDT, kind="ExternalInput").ap()
    wv_d = nc.dram_tensor("wv", [128, NFC, DH], MMDT, kind="ExternalInput").ap()
    wo_d = nc.dram_tensor("wo", [128, NOC, D], MMDT, kind="ExternalInput").ap()
    bqt_d = nc.dram_tensor("bqt", [128, NOC], F32, kind="ExternalInput").ap()
    bkt_d = nc.dram_tensor("bkt", [128, NOC], F32, kind="ExternalInput").ap()
    bvb_d = nc.dram_tensor("bvb", [128, HPC, HD], F32, kind="ExternalInput").ap()
    mask_d = nc.dram_tensor("masks", [128, 128], MMDT, kind="ExternalInput").ap()
    # fp16 partial outputs halve the output DMA traffic (the host sums the
    # two per-batch partials in fp32); quantization of an O(4) partial at
    # 2^-11 rel is ~1e-3 absolute, far under the gate
    PODT = F16 if MMDT == F16 else F32
    po_d = nc.dram_tensor("po", [S, D], PODT, kind="ExternalOutput").ap()

    with tile.TileContext(nc) as tc:
        _emit(tc, xT_d, wq_d, wk_d, wv_d, wo_d, bqt_d, bkt_d, bvb_d, mask_d,
              po_d, MMDT)
    nc.compile()
    return nc


def _emit(tc, xT_d, wq_d, wk_d, wv_d, wo_d, bqt_d, bkt_d, bvb_d, mask_d,
          po_d, MMDT):
    nc = tc.nc
    PS = bass.MemorySpace.PSUM
    PODT = po_d.dtype

    def _memset(ap, val):
        if MMDT == F32R:
            nc.vector.memset(ap.bitcast(F32), val)
        else:
            nc.vector.memset(ap, val)

    with (
        tc.tile_pool(name="persist", bufs=1) as persist,
        tc.tile_pool(name="qkv", bufs=1) as qkv_pool,
        tc.tile_pool(name="exp", bufs=3) as exp_pool,
        tc.tile_pool(name="small", bufs=2) as small_pool,
        tc.tile_pool(name="ps_mm", bufs=2, space=PS) as ps_mm,
        tc.tile_pool(name="ps_a", bufs=2, space=PS) as ps_a,
        tc.tile_pool(name="ps_ctx", bufs=2, space=PS) as ps_ctx,
    ):
        bqt = persist.tile([128, NOC], F32)
        bkt = persist.tile([128, NOC], F32)
        nc.sync.dma_start(bqt[:], bqt_d[:])
        nc.sync.dma_start(bkt[:], bkt_d[:])
        # wo is loaded during startup (scalar ring, after wq/wv) so the
        # wave-2 -> wave-3 transition never stalls on it
        wo = persist.tile([128, NOC, D], MMDT)

        # HAM pre-warm: throwaway matmuls on zeros while input DMAs land,
        # so the PE clock is at 2.4 GHz when real work starts (~3.4us of
        # sustained PE activity flips the clock gate to 8/8)
        zw = persist.tile([128, 512], MMDT)
        _memset(zw[:], 0.0)
        pwarm = ps_a.tile([128, 512], F32, name="pa")
        for i in range(14):
            _mm(nc, pwarm[:], zw[:, 0:128], zw[:], True, True)

        # one [128,128] mask serves every diagonal chunk type: with
        # cq = 128*t the causal boundary always cuts through relative
        # columns 0:128 with pattern (c >= p); duplicated across the two
        # head-halves of a score tile.  (DMA'd below, after the critical
        # weight loads.)
        mt = persist.tile([128, 2, 128], MMDT)
        bvb = persist.tile([128, HPC, HD], F32)
        ebias = persist.tile([128, 1], F32)
        nc.vector.memset(ebias[:], EXP_BIAS)

        # persistent activations.  qT doubles as ctx^T storage: wave j's
        # evacuation overwrites qT[:, :, j-tile] right after the last
        # score matmul that reads it (disjoint partition rows per head).
        qT = qkv_pool.tile([128, NOC, S], MMDT)       # q^T + bq, then ctx^T
        kT = qkv_pool.tile([128, NOC, S], MMDT)       # k^T + bk   [512, S]
        vA = qkv_pool.tile([128, NSK, HPC, 2 * HD], MMDT)  # v + 64 ones cols
        _memset(vA[:, :, :, HD:2 * HD], 1.0)

        class Feeder:
            """Spreads filler emit-callables (next-stage matmul chains)
            evenly across a wave's score chunks, so the PE always has
            independent work queued behind each exp-gated ctx matmul."""

            def __init__(self, items, total_chunks):
                self.items = list(items)
                self.total = max(1, total_chunks)
                self.done = 0
                self.fed = 0

            def chunk(self):
                want = (self.done * len(self.items)) // self.total
                while self.fed < want:
                    self.items[self.fed]()
                    self.fed += 1
                self.done += 1

            def flush(self):
                while self.fed < len(self.items):
                    self.items[self.fed]()
                    self.fed += 1

        def emit_pair(hc, j, feeder=None):
            """One (head-pair, sq-tile) attention tile.

            Heads 2*hc (partitions 0:64) and 2*hc+1 (partitions 64:128)
            share each score PSUM tile: half 0 = head A, half 1 = head B.
            The two score matmuls are adjacent in program order and sit on
            disjoint PE row-groups (tile_position (0,0) vs (64,0)), so the
            hardware runs them concurrently -- K=64 scores stop wasting
            half the array."""
            nblk = 4 * j + 4
            ctxpA = ps_ctx.tile([128, 512], F32, name="ctxp")
            ctxpB = ps_ctx.tile([128, 512], F32, name="ctxp")
            qTjA = qT[0:64, hc, ts(j, 512)]
            qTjB = qT[64:128, hc, ts(j, 512)]

            for ik in range(nblk):
                if feeder is not None:
                    feeder.chunk()
                t = ik - 4 * j
                cq = 128 * t if t >= 0 else 0
                w = 512 - cq
                stp = ps_mm.tile([128, 2, 512], F32, name="mm")
                _mm(nc, stp[:, 0, 0:w],
                    kT[0:64, hc, ts(ik, 128)], qTjA[:, cq:512], True, True)
                _mm(nc, stp[:, 1, 0:w],
                    kT[64:128, hc, ts(ik, 128)], qTjB[:, cq:512], True, True)
                e = exp_pool.tile([128, 2, 512], MMDT, name="e")
                nc.scalar.activation(e[:, :, 0:w], stp[:, :, 0:w],
                                     AF.Exp, scale=0.125, bias=ebias[:])
                if t >= 0:
                    # the causal boundary always lies in relative cols 0:128
                    nc.vector.tensor_mul(e[:, :, 0:128],
                                         e[:, :, 0:128], mt[:])
                _mm(nc, ctxpA[:, cq:512], vA[:, ik, 2 * hc, :],
                    e[:, 0, 0:w], ik == 0, ik == nblk - 1)
                _mm(nc, ctxpB[:, cq:512], vA[:, ik, 2 * hc + 1, :],
                    e[:, 1, 0:w], ik == 0, ik == nblk - 1)
            # normalize straight out of PSUM into qT's freed j-tile: the
            # denominator arrives replicated on partitions 64:128
            for half, ctxp in ((0, ctxpA), (1, ctxpB)):
                hp = half * 64
                dst = qT[hp:hp + 64, hc, ts(j, 512)]
                # custom-DVE ops can't read PSUM; stage the replicated den
                # rows through SBUF with a plain copy first
                den = small_pool.tile([64, 512], F32, name="den")
                nc.vector.tensor_copy(den[:], ctxp[64:128, :])
                rec = small_pool.tile([64, 512], F32, name="rec")
                nc.vector.reciprocal_approx_fast(rec[:], den[:])
                nc.vector.tensor_mul(dst, ctxp[0:64, :], rec[:])

        with (
            tc.tile_pool(name="wtsA", bufs=1) as wtsA,
            tc.tile_pool(name="xin", bufs=2) as xin,
        ):
            xts = {}
            xts[0] = xin.tile([128, NFC, 512], MMDT, name="xt")
            wq = wtsA.tile([128, NFC, DH], MMDT)
            wk = wtsA.tile([128, NFC, DH], MMDT)
            wv = wtsA.tile([128, NFC, DH], MMDT)
            # one big fully-contiguous DMA per tensor (splits across all 16
            # SDMA engines), spread over BOTH HWDGE rings in the order the
            # A(0) units consume them: sync carries x then wk, scalar (idle
            # at startup) carries wq then wv then wo
            nc.sync.dma_start(xts[0][:], xT_d[0])
            nc.scalar.dma_start(wq[:], wq_d[:])
            nc.sync.dma_start(wk[:], wk_d[:])
            nc.scalar.dma_start(wv[:], wv_d[:])
            nc.scalar.dma_start(wo[:], wo_d[:])
            nc.sync.dma_start(bvb[:], bvb_d[:])
            for hh in range(2):
                nc.scalar.dma_start(mt[:, hh], mask_d[:])

            def A_half_units(j):
                """Stage A(j) as 12 independently-emittable half-chain
                units (8 matmuls + evacuation each), for weaving into the
                previous wave at chunk granularity via Feeder."""
                units = []
                for g in range(6):
                    for half in range(2):
                        def unit(g=g, half=half):
                            pt = ps_a.tile([128, 512], F32, name="pa")
                            if g < 4:
                                op, is_k = g // 2, g % 2
                                wt, bias, dstT = ((wk, bkt, kT) if is_k
                                                  else (wq, bqt, qT))
                                oc = 2 * op + half
                                for fc in range(NFC):
                                    _mm(nc, pt[:],
                                        wt[:, fc, ts(oc, 128)],
                                        xts[j][:, fc],
                                        fc == 0, fc == NFC - 1)
                                # DVE bias-add evac keeps the wave's ACT
                                # stream pure exp (exp rate paces the wave)
                                nc.vector.tensor_scalar_add(
                                    dstT[:, oc, ts(j, 512)], pt[:],
                                    bias[:, oc:oc + 1])
                            else:
                                sc = 2 * (g - 4) + half
                                for fc in range(NFC):
                                    _mm(nc, pt[:],
                                        xts[j][:, fc, ts(sc, 128)],
                                        wv[:, fc],
                                        fc == 0, fc == NFC - 1)
                                pv_r = pt[:].rearrange("p (h u) -> p h u",
                                                       u=HD)
                                nc.vector.tensor_add(
                                    vA[:, 4 * j + sc, :, 0:HD], pv_r,
                                    bvb[:])
                        units.append(unit)
                return units

            # A(0) is emitted just-in-time around wave 0's pairs: pair
            # (hc, 0) needs only q/k chains for out-chunk hc plus all four
            # v units, so wave 0's exp work starts ~10us earlier and
            # overlaps the back half of A(0).  A(1) weaves into wave 0 via
            # the feeder as before; waves 1-2 carry A(2)/A(3).
            u0 = A_half_units(0)
            # unit index by (g, half): q oc: g in {0, 2}, k oc: g in {1, 3}
            q_of = {0: u0[0], 1: u0[1], 2: u0[4], 3: u0[5]}
            k_of = {0: u0[2], 1: u0[3], 2: u0[6], 3: u0[7]}
            v_us = u0[8:12]
            xts[1] = xin.tile([128, NFC, 512], MMDT, name="xt")
            nc.sync.dma_start(xts[1][:], xT_d[1])
            q_of[0]()
            k_of[0]()
            for u in v_us:
                u()
            feeder = Feeder(A_half_units(1), 16)
            for hc in range(NOC):
                if hc > 0:
                    q_of[hc]()
                    k_of[hc]()
                emit_pair(hc, 0, feeder)
            feeder.flush()
            for j in range(1, 3):
                xts[j + 1] = xin.tile([128, NFC, 512], MMDT, name="xt")
                nc.sync.dma_start(xts[j + 1][:], xT_d[j + 1])
                feeder = Feeder(A_half_units(j + 1), 4 * (4 * j + 4))
                for hc in range(NOC):
                    emit_pair(hc, j, feeder)
                feeder.flush()

        # wave 3 with stage-C groups for sq-chunks 0..11 woven in
        # (they only need waves 0-2), then the 4-group C tail
        with (
            tc.tile_pool(name="poC", bufs=3) as poC,
        ):
            def emit_C_group(sq):
                # C chains use the ps_a pool (stage A is done by wave 3)
                # so weaving them between score chunks never blocks the
                # score-tile rotation in ps_mm
                ot = poC.tile([128, 2, 512], PODT, name="ot")
                for oc in range(2):
                    pp = ps_a.tile([128, 512], F32, name="pa")
                    for hc in range(NOC):
                        _mm(nc, pp[:], qT[:, hc, ts(sq, 128)],
                            wo[:, hc, ts(oc, 512)],
                            hc == 0, hc == NOC - 1)
                    nc.vector.tensor_copy(ot[:, oc, :], pp[:])
                nc.sync.dma_start(
                    po_d[ts(sq, 128), :],
                    ot[:].rearrange("p a b -> p (a b)"))

            feeder = Feeder([lambda sq=sq: emit_C_group(sq)
                             for sq in range(12)], 64)
            for hc in range(NOC):
                emit_pair(hc, 3, feeder)
            feeder.flush()
            # keep-warm filler: the last pair's normalization (DVE) gates
            # the C tail; ~3us of PE idle here would re-throttle the clock
            # to 1.2 GHz and double the tail's matmul time
            for b in range(2):
                dw = ps_a.tile([128, 512], F32, name="pa")
                for i in range(8):
                    _mm(nc, dw[:], zw[:, 0:128], zw[:], True, True)
            for sq in range(12, 16):
                emit_C_group(sq)


def make_mask():
    p = np.arange(128)[:, None]
    c = np.arange(128)[None, :]
    return (c >= p).astype(np.float32)


def _tile_w(W, nfc):
    """[nfc*128, dout] -> [128, nfc, dout] (pre-tiled SBUF layout)."""
    return np.ascontiguousarray(
        W.reshape(nfc, 128, W.shape[1]).transpose(1, 0, 2))


def make_in_maps(x, Wq, bq, Wk, bk, Wv, bv, Wo, mode="fp16"):
    npdt = _NPDT[mode]
    mask = make_mask().astype(npdt)
    in_maps = []
    xts = {}
    for b in range(B):
        # xT[j, p, f, s] = x[b][j*512+s, f*128+p]
        xb = x[b].astype(npdt)
        xts[b] = np.ascontiguousarray(
            xb.reshape(NSQ, 512, NFC, 128).transpose(0, 3, 2, 1))
    for c in range(NCORES):
        b, g = c // 2, c % 2
        sl = slice(g * DH, (g + 1) * DH)
        in_maps.append({
            "xT": xts[b],
            "wq": _tile_w(Wq[:, sl].astype(npdt), NFC),
            "wk": _tile_w(Wk[:, sl].astype(npdt), NFC),
            "wv": _tile_w(Wv[:, sl].astype(npdt), NFC),
            "wo": _tile_w(Wo[sl, :].astype(npdt), NOC),
            "bqt": np.ascontiguousarray(bq[sl].reshape(NOC, 128).T),
            "bkt": np.ascontiguousarray(bk[sl].reshape(NOC, 128).T),
            "bvb": np.ascontiguousarray(
                np.broadcast_to(bv[sl].reshape(HPC, HD), (128, HPC, HD))),
            "masks": mask,
        })
    return in_maps


_CACHE = {}


def _get_program(mode="fp16"):
    key = ("prog", mode)
    if key not in _CACHE:
        _CACHE[key] = build_program(mode=mode)
    return _CACHE[key]


def kernel(x, Wq, bq, Wk, bk, Wv, bv, Wo, bo, **run_kwargs):
    x = np.asarray(x, dtype=np.float32)
    Wq = np.asarray(Wq, dtype=np.float32)
    bq = np.asarray(bq, dtype=np.float32)
    Wk = np.asarray(Wk, dtype=np.float32)
    bk = np.asarray(bk, dtype=np.float32)
    Wv = np.asarray(Wv, dtype=np.float32)
    bv = np.asarray(bv, dtype=np.float32)
    Wo = np.asarray(Wo, dtype=np.float32)
    bo = np.asarray(bo, dtype=np.float32)

    mode = run_kwargs.pop("mode", "fp16")
    nc = _get_program(mode=mode)
    in_maps = make_in_maps(x, Wq, bq, Wk, bk, Wv, bv, Wo, mode=mode)
    res = run_bass_kernel_spmd(nc, in_maps, list(range(NCORES)), **run_kwargs)
    out = np.empty((B, S, D), dtype=np.float32)
    for b in range(B):
        out[b] = (res.results[2 * b]["po"].astype(np.float32)
                  + res.results[2 * b + 1]["po"].astype(np.float32) + bo)
    _CACHE["last_results"] = res
    return out


# revision 30
# speedup vs baseline: 1.0509x; 1.0509x over previous
"""Multi-head masked self-attention on 8 TRN2 NeuronCores.

Problem: B=4, S=2048, D=1024, H=16 heads (hd=64), fp32.
  q,k,v = x@W* + b*; causal softmax(q k^T / 8) @ v; out = ctx @ Wo + bo.

Sharding: core c -> (batch b = c//2, head-group g = c%2 of 8 heads).
Each core computes a partial output projection over its 512 hidden dims;
the host sums the two partials per batch and adds bo.

On-device layout strategy (no on-device transposes needed):
  - host passes xT = x[b].T  [D, S]
  - q^T, k^T computed directly as [512, S] (lhsT = W chunk, rhs = xT chunk)
  - v computed in natural [S, 512] layout (lhsT = xT chunk, rhs = Wv chunk),
    stored interleaved with 64 ones-columns per head ("v_aug", [S, 8*128]):
    the ctx matmul then accumulates the softmax denominator REPLICATED on
    PSUM partitions 64:128 for free (matmul cost depends only on the
    streamed column count N, not the stationary width M), so normalization
    is just a [64,512] reciprocal + multiply on DVE -- no gpsimd
    partition_broadcast, no denominator copy
  - scores are computed transposed: st[sk, sq] = k q^T; exp via ACT with the
    free affine bias: e = exp(s/8 - 2).  The -2 keeps e inside fp16 range
    (max score/8 measured ~8.8 -> e^6.8 ~ 900) and cancels exactly in the
    softmax normalization (numerator and denominator share the factor).
  - every diagonal chunk t=ik-4j in {0,1,2,3} computes only the columns the
    causal boundary allows (cq = 128*t), and in that frame the boundary
    always cuts through relative columns 0:128 with the SAME pattern
    (c >= p), so one [128,128] mask serves all four chunk types
  - heads are processed in PAIRS (partitions 0:64 / 64:128): the two K=64
    score matmuls per sk-chunk share one PSUM tile (separate banks) and are
    adjacent in program order, so the PE runs them concurrently on disjoint
    row-groups (tile_position (0,0)/(64,0)) -- 2x score throughput
  - ctx^T[hd, sq] accumulated in PSUM = v_aug^T.T @ exp; normalization:
    fast approximate reciprocal of the den row on DVE, gpsimd
    partition_broadcast, multiply on DVE during evacuation
  - output projection uses ctx^T directly as lhsT (again no transpose);
    ctx^T aliases qT's storage (each qT j-tile dies as its wave completes)
  - all matmul operands are float16 (true 1 col/cycle streaming + FWL
    weight loads, unlike f32r whose fp32_mode=HIGH path measures ~1.8x
    slower per matmul and disables FWL); accumulation stays fp32 in PSUM.
    fp16 keeps ~11 bits of mantissa -> rel err ~1e-3, far under the 2e-2
    gate.
  - PSUM pools: scores/C 2x[128,2,512], A-chains 2x[128,512] (dedicated so
    pool rotation never lets attention stall the projections), ctx 2x[65,512]
  - stage A(j+1) is cut into 12 half-chain units and WOVEN between wave j's
    score chunks (Feeder); likewise C(sq<12) weaves into wave 3.  This keeps
    independent PE work queued behind every exp-gated ctx matmul, which keeps
    PE duty high enough that the HAM clock stays at 2.4 GHz for the whole
    middle of the kernel (HAM re-throttles to 1.2 GHz after idle windows)
"""

import numpy as np

import concourse.bass as bass
import concourse.mybir as mybir
import concourse.tile as tile
from concourse import bacc
from concourse.bass import ts
from concourse.bass_utils import run_bass_kernel_spmd

F32 = mybir.dt.float32
F32R = mybir.dt.float32r
F16 = mybir.dt.float16
AF = mybir.ActivationFunctionType

B, S, D, H, HD = 4, 2048, 1024, 16, 64
G = 2                 # head groups (cores per batch)
DH = D // G           # hidden dims per core = 512
HPC = H // G          # heads per core = 8
NCORES = 8

NSQ = S // 512        # 4 sq tiles of 512
NSK = S // 128        # 16 sk chunks of 128
NFC = D // 128        # 8 feature chunks
NOC = DH // 128       # 4 out-dim chunks of the per-core hidden

EXP_BIAS = -2.0       # e = exp(s/8 + EXP_BIAS); cancels in normalization

_DT = {"fp16": F16, "f32r": F32R, "f32": F32}
_NPDT = {"fp16": np.float16, "f32r": np.float32, "f32": np.float32}


def _mm(nc, out, lhsT, rhs, start, stop):
    nc.tensor.matmul(out, lhsT, rhs, start=start, stop=stop)


def build_program(mode="fp16"):
    """Build the single-core SPMD Bass program (same program on all 8 cores)."""
    nc = bacc.Bacc("TRN2", target_bir_lowering=False, debug=False)
    MMDT = _DT[mode]  # dtype of every matmul operand

    # all large inputs arrive pre-tiled by the host into the exact SBUF
    # layout, so every load is one fully-contiguous DMA at line rate
    xT_d = nc.dram_tensor("xT", [NSQ, 128, NFC, 512], MMDT,
                          kind="ExternalInput").ap()
    wq_d = nc.dram_tensor("wq", [128, NFC, DH], MMDT, kind="ExternalInput").ap()
    wk_d = nc.dram_tensor("wk", [128, NFC, DH], MMDT, kind="ExternalInput").ap()
    wv_d = nc.dram_tensor("wv", [128, NFC, DH], MMDT, kind="ExternalInput").ap()
    wo_d = nc.dram_tensor("wo", [128, NOC, D], MMDT, kind="ExternalInput").ap()
    bqt_d = nc.dram_tensor("bqt", [128, NOC], F32, kind="ExternalInput").ap()
    bkt_d = nc.dram_tensor("bkt", [128, NOC], F32, kind="ExternalInput").ap()
    bvb_d = nc.dram_tensor("bvb", [128, HPC, HD], F32, kind="ExternalInput").ap()
    mask_d = nc.dram_tensor("masks", [128, 128], MMDT, kind="ExternalInput").ap()
    # fp16 partial outputs halve the output DMA traffic (the host sums the
    # two per-batch partials in fp32); quantization of an O(4) partial at
    # 2^-11 rel is ~1e-3 absolute, far under the gate
    PODT = F16 if MMDT == F16 else F32
    po_d = nc.dram_tensor("po", [S, D], PODT, kind="ExternalOutput").ap()

    with tile.TileContext(nc) as tc:
        _emit(tc, xT_d, wq_d, wk_d, wv_d, wo_d, bqt_d, bkt_d, bvb_d, mask_d,
              po_d, MMDT)
    nc.compile()
    return nc


def _emit(tc, xT_d, wq_d, wk_d, wv_d, wo_d, bqt_d, bkt_d, bvb_d, mask_d,
          po_d, MMDT):
    nc = tc.nc
    PS = bass.MemorySpace.PSUM
    PODT = po_d.dtype

    def _memset(ap, val):
        if MMDT == F32R:
            nc.vector.memset(ap.bitcast(F32), val)
        else:
            nc.vector.memset(ap, val)

    with (
        tc.tile_pool(name="persist", bufs=1) as persist,
        tc.tile_pool(name="qkv", bufs=1) as qkv_pool,
        tc.tile_pool(name="exp", bufs=3) as exp_pool,
        tc.tile_pool(name="small", bufs=2) as small_pool,
        tc.tile_pool(name="ps_mm", bufs=2, space=PS) as ps_mm,
        tc.tile_pool(name="ps_a", bufs=2, space=PS) as ps_a,
        tc.tile_pool(name="ps_ctx", bufs=2, space=PS) as ps_ctx,
    ):
        bqt = persist.tile([128, NOC], F32)
        bkt = persist.tile([128, NOC], F32)
        nc.sync.dma_start(bqt[:], bqt_d[:])
        nc.sync.dma_start(bkt[:], bkt_d[:])
        # wo is loaded during startup (scalar ring, after wq/wv) so the
        # wave-2 -> wave-3 transition never stalls on it
        wo = persist.tile([128, NOC, D], MMDT)

        # HAM pre-warm: throwaway matmuls on zeros while input DMAs land,
        # so the PE clock is at 2.4 GHz when real work starts (~3.4us of
        # sustained PE activity flips the clock gate to 8/8)
        zw = persist.tile([128, 512], MMDT)
        _memset(zw[:], 0.0)
        pwarm = ps_a.tile([128, 512], F32, name="pa")
        for i in range(14):
            _mm(nc, pwarm[:], zw[:, 0:128], zw[:], True, True)

        # one [128,128] mask serves every diagonal chunk type: with
        # cq = 128*t the causal boundary always cuts through relative
        # columns 0:128 with pattern (c >= p); duplicated across the two
        # head-halves of a score tile.  (DMA'd below, after the critical
        # weight loads.)
        mt = persist.tile([128, 2, 128], MMDT)
        bvb = persist.tile([128, HPC, HD], F32)
        ebias = persist.tile([128, 1], F32)
        nc.vector.memset(ebias[:], EXP_BIAS)

        # persistent activations.  qT doubles as ctx^T storage: wave j's
        # evacuation overwrites qT[:, :, j-tile] right after the last
        # score matmul that reads it (disjoint partition rows per head).
        qT = qkv_pool.tile([128, NOC, S], MMDT)       # q^T + bq, then ctx^T
        kT = qkv_pool.tile([128, NOC, S], MMDT)       # k^T + bk   [512, S]
        vA = qkv_pool.tile([128, NSK, HPC, 2 * HD], MMDT)  # v + 64 ones cols
        _memset(vA[:, :, :, HD:2 * HD], 1.0)

        class Feeder:
            """Spreads filler emit-callables (next-stage matmul chains)
            evenly across a wave's score chunks, so the PE always has
            independent work queued behind each exp-gated ctx matmul."""

            def __init__(self, items, total_chunks):
                self.items = list(items)
                self.total = max(1, total_chunks)
                self.done = 0
                self.fed = 0

            def chunk(self):
                want = (self.done * len(self.items)) // self.total
                while self.fed < want:
                    self.items[self.fed]()
                    self.fed += 1
                self.done += 1

            def flush(self):
                while self.fed < len(self.items):
                    self.items[self.fed]()
                    self.fed += 1

        def emit_pair(hc, j, feeder=None):
            """One (head-pair, sq-tile) attention tile.

            Heads 2*hc (partitions 0:64) and 2*hc+1 (partitions 64:128)
            share each score PSUM tile: half 0 = head A, half 1 = head B.
            The two score matmuls are adjacent in program order and sit on
            disjoint PE row-groups (tile_position (0,0) vs (64,0)), so the
            hardware runs them concurrently -- K=64 scores stop wasting
            half the array."""
            nblk = 4 * j + 4
            ctxpA = ps_ctx.tile([128, 512], F32, name="ctxp")
            ctxpB = ps_ctx.tile([128, 512], F32, name="ctxp")
            qTjA = qT[0:64, hc, ts(j, 512)]
            qTjB = qT[64:128, hc, ts(j, 512)]

            for ik in range(nblk):
                if feeder is not None:
                    feeder.chunk()
                t = ik - 4 * j
                cq = 128 * t if t >= 0 else 0
                w = 512 - cq
                stp = ps_mm.tile([128, 2, 512], F32, name="mm")
                _mm(nc, stp[:, 0, 0:w],
                    kT[0:64, hc, ts(ik, 128)], qTjA[:, cq:512], True, True)
                _mm(nc, stp[:, 1, 0:w],
                    kT[64:128, hc, ts(ik, 128)], qTjB[:, cq:512], True, True)
                e = exp_pool.tile([128, 2, 512], MMDT, name="e")
                nc.scalar.activation(e[:, :, 0:w], stp[:, :, 0:w],
                                     AF.Exp, scale=0.125, bias=ebias[:])
                if t >= 0:
                    # the causal boundary always lies in relative cols 0:128
                    nc.vector.tensor_mul(e[:, :, 0:128],
                                         e[:, :, 0:128], mt[:])
                _mm(nc, ctxpA[:, cq:512], vA[:, ik, 2 * hc, :],
                    e[:, 0, 0:w], ik == 0, ik == nblk - 1)
                _mm(nc, ctxpB[:, cq:512], vA[:, ik, 2 * hc + 1, :],
                    e[:, 1, 0:w], ik == 0, ik == nblk - 1)
            # normalize straight out of PSUM into qT's freed j-tile: the
            # denominator arrives replicated on partitions 64:128
            for half, ctxp in ((0, ctxpA), (1, ctxpB)):
                hp = half * 64
                dst = qT[hp:hp + 64, hc, ts(j, 512)]
                # custom-DVE ops can't read PSUM; stage the replicated den
                # rows through SBUF with a plain copy first
                den = small_pool.tile([64, 512], F32, name="den")
                nc.vector.tensor_copy(den[:], ctxp[64:128, :])
                rec = small_pool.tile([64, 512], F32, name="rec")
                nc.vector.reciprocal_approx_fast(rec[:], den[:])
                nc.vector.tensor_mul(dst, ctxp[0:64, :], rec[:])

        with (
            tc.tile_pool(name="wtsA", bufs=1) as wtsA,
            tc.tile_pool(name="xin", bufs=2) as xin,
        ):
            xts = {}
            xts[0] = xin.tile([128, NFC, 512], MMDT, name="xt")
            wq = wtsA.tile([128, NFC, DH], MMDT)
            wk = wtsA.tile([128, NFC, DH], MMDT)
            wv = wtsA.tile([128, NFC, DH], MMDT)
            # one big fully-contiguous DMA per tensor (splits across all 16
            # SDMA engines), spread over BOTH HWDGE rings in the order the
            # A(0) units consume them: sync carries x then wk, scalar (idle
            # at startup) carries wq then wv then wo
            nc.sync.dma_start(xts[0][:], xT_d[0])
            nc.scalar.dma_start(wq[:], wq_d[:])
            nc.sync.dma_start(wk[:], wk_d[:])
            nc.scalar.dma_start(wv[:], wv_d[:])
            nc.scalar.dma_start(wo[:], wo_d[:])
            nc.sync.dma_start(bvb[:], bvb_d[:])
            for hh in range(2):
                nc.scalar.dma_start(mt[:, hh], mask_d[:])

            def A_half_units(j):
                """Stage A(j) as 12 independently-emittable half-chain
                units (8 matmuls + evacuation each), for weaving into the
                previous wave at chunk granularity via Feeder."""
                units = []
                for g in range(6):
                    for half in range(2):
                        def unit(g=g, half=half):
                            pt = ps_a.tile([128, 512], F32, name="pa")
                            if g < 4:
                                op, is_k = g // 2, g % 2
                                wt, bias, dstT = ((wk, bkt, kT) if is_k
                                                  else (wq, bqt, qT))
                                oc = 2 * op + half
                                for fc in range(NFC):
                                    _mm(nc, pt[:],
                                        wt[:, fc, ts(oc, 128)],
                                        xts[j][:, fc],
                                        fc == 0, fc == NFC - 1)
                                # DVE bias-add evac keeps the wave's ACT
                                # stream pure exp (exp rate paces the wave)
                                nc.vector.tensor_scalar_add(
                                    dstT[:, oc, ts(j, 512)], pt[:],
                                    bias[:, oc:oc + 1])
                            else:
                                sc = 2 * (g - 4) + half
                                for fc in range(NFC):
                                    _mm(nc, pt[:],
                                        xts[j][:, fc, ts(sc, 128)],
                                        wv[:, fc],
                                        fc == 0, fc == NFC - 1)
                                pv_r = pt[:].rearrange("p (h u) -> p h u",
                                                       u=HD)
                                nc.vector.tensor_add(
                                    vA[:, 4 * j + sc, :, 0:HD], pv_r,
                                    bvb[:])
                        units.append(unit)
                return units

            # A(0) runs up front, reordered to match DMA arrival (wq lands
            # first, then wk, then wv: all q units, then k, then v); each
            # wave j then carries A(j+1) woven between its score chunks so
            # the PE has independent fill work behind every exp-gated ctx
            # matmul
            u0 = A_half_units(0)
            for i in [0, 1, 4, 5, 2, 3, 6, 7, 8, 9, 10, 11]:
                u0[i]()
            for j in range(3):
                xts[j + 1] = xin.tile([128, NFC, 512], MMDT, name="xt")
                nc.sync.dma_start(xts[j + 1][:], xT_d[j + 1])
                feeder = Feeder(A_half_units(j + 1), 4 * (4 * j + 4))
                for hc in range(NOC):
                    emit_pair(hc, j, feeder)
                feeder.flush()

        # wave 3 with stage-C groups for sq-chunks 0..11 woven in
        # (they only need waves 0-2), then the 4-group C tail
        with (
            tc.tile_pool(name="poC", bufs=3) as poC,
        ):
            def emit_C_group(sq):
                # C chains use the ps_a pool (stage A is done by wave 3)
                # so weaving them between score chunks never blocks the
                # score-tile rotation in ps_mm
                ot = poC.tile([128, 2, 512], PODT, name="ot")
                for oc in range(2):
                    pp = ps_a.tile([128, 512], F32, name="pa")
                    for hc in range(NOC):
                        _mm(nc, pp[:], qT[:, hc, ts(sq, 128)],
                            wo[:, hc, ts(oc, 512)],
                            hc == 0, hc == NOC - 1)
                    nc.vector.tensor_copy(ot[:, oc, :], pp[:])
                nc.sync.dma_start(
                    po_d[ts(sq, 128), :],
                    ot[:].rearrange("p a b -> p (a b)"))

            feeder = Feeder([lambda sq=sq: emit_C_group(sq)
                             for sq in range(12)], 64)
            for hc in range(NOC):
                emit_pair(hc, 3, feeder)
            feeder.flush()
            # keep-warm filler: the last pair's normalization (DVE) gates
            # the C tail; ~3us of PE idle here would re-throttle the clock
            # to 1.2 GHz and double the tail's matmul time
            for b in range(2):
                dw = ps_a.tile([128, 512], F32, name="pa")
                for i in range(8):
                    _mm(nc, dw[:], zw[:, 0:128], zw[:], True, True)
            for sq in range(12, 16):
                emit_C_group(sq)


def make_mask():
    p = np.arange(128)[:, None]
    c = np.arange(128)[None, :]
    return (c >= p).astype(np.float32)


def _tile_w(W, nfc):
    """[nfc*128, dout] -> [128, nfc, dout] (pre-tiled SBUF layout)."""
    return np.ascontiguousarray(
        W.reshape(nfc, 128, W.shape[1]).transpose(1, 0, 2))


def make_in_maps(x, Wq, bq, Wk, bk, Wv, bv, Wo, mode="fp16"):
    npdt = _NPDT[mode]
    mask = make_mask().astype(npdt)
    in_maps = []
    xts = {}
    for b in range(B):
        # xT[j, p, f, s] = x[b][j*512+s, f*128+p]
        xb = x[b].astype(npdt)
        xts[b] = np.ascontiguousarray(
            xb.reshape(NSQ, 512, NFC, 128).transpose(0, 3, 2, 1))
    for c in range(NCORES):
        b, g = c // 2, c % 2
        sl = slice(g * DH, (g + 1) * DH)
        in_maps.append({
            "xT": xts[b],
            "wq": _tile_w(Wq[:, sl].astype(npdt), NFC),
            "wk": _tile_w(Wk[:, sl].astype(npdt), NFC),
            "wv": _tile_w(Wv[:, sl].astype(npdt), NFC),
            "wo": _tile_w(Wo[sl, :].astype(npdt), NOC),
            "bqt": np.ascontiguousarray(bq[sl].reshape(NOC, 128).T),
            "bkt": np.ascontiguousarray(bk[sl].reshape(NOC, 128).T),
            "bvb": np.ascontiguousarray(
                np.broadcast_to(bv[sl].reshape(HPC, HD), (128, HPC, HD))),
            "masks": mask,
        })
    return in_maps


_CACHE = {}


def _get_program(mode="fp16"):
    key = ("prog", mode)
    if key not in _CACHE:
        _CACHE[key] = build_program(mode=mode)
    return _CACHE[key]


def kernel(x, Wq, bq, Wk, bk, Wv, bv, Wo, bo, **run_kwargs):
    x = np.asarray(x, dtype=np.float32)
    Wq = np.asarray(Wq, dtype=np.float32)
    bq = np.asarray(bq, dtype=np.float32)
    Wk = np.asarray(Wk, dtype=np.float32)
    bk = np.asarray(bk, dtype=np.float32)
    Wv = np.asarray(Wv, dtype=np.float32)
    bv = np.asarray(bv, dtype=np.float32)
    Wo = np.asarray(Wo, dtype=np.float32)
    bo = np.asarray(bo, dtype=np.float32)

    mode = run_kwargs.pop("mode", "fp16")
    nc = _get_program(mode=mode)
    in_maps = make_in_maps(x, Wq, bq, Wk, bk, Wv, bv, Wo, mode=mode)
    res = run_bass_kernel_spmd(nc, in_maps, list(range(NCORES)), **run_kwargs)
    out = np.empty((B, S, D), dtype=np.float32)
    for b in range(B):
        out[b] = (res.results[2 * b]["po"].astype(np.float32)
                  + res.results[2 * b + 1]["po"].astype(np.float32) + bo)
    _CACHE["last_results"] = res
    return out
